# revision 10
# baseline (speedup 1.0000x reference)
"""Causal self-attention (B=4, T=2048, C=1024, H=16) on 8 TRN2 NeuronCores.

Sharding: hybrid batch x head tensor-parallel. Core c handles batch b = c//2
and heads [8*(c%2) : 8*(c%2)+8]. Each core computes QKV for its 8 heads over
its batch, full causal attention for those heads, and a *partial* c_proj
(contribution of its 8 heads to all 2048 tokens of its batch). The host
unshards by summing the two partial outputs of each batch pair; b_proj is
added on-device by the even core of each pair.

Single-pass pipelined structure (per core): one loop over the 4 q-windows of
512 tokens. Per window w: QK projections for that token window (Q kept only
for the window, K appended to a persistent K_T), V for the window's 4 token
blocks, then causal attention for all 8 heads over k-blocks 0..4w+3 (scores
matmul -> exp on Act -> triangular mask on DVE -> attn@V accumulate), per-head
softmax normalization (denominator rides in PSUM row 64 via a ones column in
the V tiles), then the window's partial c_proj with the bias folded in as a
rank-1 accumulate matmul and gpsimd moving PSUM->SBUF for the output DMA.
x is fed transposed (xt [C, T]) and loaded once per window; weights stay
resident in SBUF. All matmuls are float32r (full-rate fp32, moving dim kept
>= 256 everywhere: the 128-wide diagonal chunks are widened to 256 with a
zero-extended triangular mask).
"""

import numpy as np

import concourse.bass as bass
import concourse.mybir as mybir
import concourse.tile as tile
from concourse import bacc
from concourse.bass_utils import run_bass_kernel_spmd

B, T, C = 4, 2048, 1024
H = 16          # total heads
HL = 8          # heads per core
D = 64          # head dim
P = 128
W = 512         # q-window / matmul moving-dim size
NW = T // W     # 4 q windows
KB = T // P     # 16 k blocks
NCHUNK = C // P  # 8 contraction chunks over C
PAIRS = HL // 2  # 4 head-pairs (2 heads per 128-partition tile)
F32 = mybir.dt.float32
F32R = mybir.dt.float32r
EXP = mybir.ActivationFunctionType.Exp
N_CORES = 8
LAG = 2          # scores->attn@V software pipeline depth per head

_CACHE = {}
LAST_RESULTS = None


def build_nc():
    if "nc" in _CACHE:
        return _CACHE["nc"]
    nc = bacc.Bacc(
        "TRN2", target_bir_lowering=False, debug=False, num_devices=N_CORES
    )

    xt = nc.dram_tensor("xt", [C, T], F32R, kind="ExternalInput")
    wqk = nc.dram_tensor("wqk", [C, C], F32R, kind="ExternalInput")
    wv = nc.dram_tensor("wv", [C, HL * D], F32R, kind="ExternalInput")
    bqk = nc.dram_tensor("bqk", [P, 2 * PAIRS], F32, kind="ExternalInput")
    bv = nc.dram_tensor("bv", [P, HL * D], F32, kind="ExternalInput")
    wp = nc.dram_tensor("wp", [HL * D, C], F32R, kind="ExternalInput")
    bpr = nc.dram_tensor("bpr", [P, C], F32, kind="ExternalInput")
    trimask = nc.dram_tensor("trimask", [P, P], F32, kind="ExternalInput")
    trimask2 = nc.dram_tensor("trimask2", [P, 2 * P], F32, kind="ExternalInput")
    onesd = nc.dram_tensor("onesd", [P, P], F32R, kind="ExternalInput")
    out = nc.dram_tensor("out", [T, C], F32, kind="ExternalOutput")

    xt_r = xt[:].rearrange("(a p) t -> p a t", p=P)

    with tile.TileContext(nc) as tc, nc.allow_low_precision(
        reason="float32r tiles for full-rate fp32 PE matmuls"
    ):
        with (
            tc.tile_pool(name="consts", bufs=1) as consts,
            tc.tile_pool(name="waqk", bufs=NCHUNK) as waqk_pool,
            tc.tile_pool(name="wav", bufs=NCHUNK) as wav_pool,
            tc.tile_pool(name="xtw", bufs=1) as xtw_pool,
            tc.tile_pool(name="kt", bufs=1) as kt_pool,
            tc.tile_pool(name="qt", bufs=PAIRS) as qt_pool,
            tc.tile_pool(name="vsb", bufs=1) as v_pool,
            tc.tile_pool(name="attn", bufs=LAG + 1) as attn_pool,
            tc.tile_pool(name="yt", bufs=1) as yt_pool,
            tc.tile_pool(name="wp_sb", bufs=1) as wp_pool,
            tc.tile_pool(name="osb", bufs=2) as o_pool,
            tc.tile_pool(name="norm", bufs=1) as norm_pool,
            tc.tile_pool(name="psum", space="PSUM", bufs=4) as psum,
        ):
            # ---- const tiles
            bqk_t = consts.tile([P, 2 * PAIRS], F32)
            bv_t = consts.tile([P, HL * D], F32)
            tri_t = consts.tile([P, P], F32)
            tri2_t = consts.tile([P, 2 * P], F32)
            ones_row = consts.tile([1, P], F32R)
            bpr_t = consts.tile([P, C], F32)

            waqk_sb = [
                waqk_pool.tile([P, C], F32R, tag="waqk", name=f"waqk{a}")
                for a in range(NCHUNK)
            ]
            wav_sb = [
                wav_pool.tile([P, HL * D], F32R, tag="wav", name=f"wav{a}")
                for a in range(NCHUNK)
            ]
            kt_sb = [
                kt_pool.tile([P, T], F32R, tag=f"kt{pr}", name=f"kt{pr}")
                for pr in range(PAIRS)
            ]
            wp_sb = [
                wp_pool.tile([P, C], F32R, tag=f"wp{ch}", name=f"wp{ch}")
                for ch in range(PAIRS)
            ]
            # V laid out [tok, d] per (head, kblock) as [P, 65] slices
            # (col 64 stays 1.0 so attn@V accumulates softmax denominators).
            v_sb = v_pool.tile([P, HL * KB * 65], F32R)
            v_view = v_sb[:].rearrange("p (h k c) -> p h k c", h=HL, k=KB)
            bv_view = bv_t[:].rearrange("p (h d) -> p h d", h=HL)

            # ---- DMA kickoff, window-0 critical path first: Q-half weight
            # chunks interleaved with xt window-0 chunks, then K halves,
            # then wv; everything else after.
            xtw_tiles = {}

            def xtw_get(w):
                if w not in xtw_tiles:
                    t = xtw_pool.tile([P, NCHUNK * W], F32R, tag="xtw",
                                      name=f"xtw{w}")
                    tv = t[:].rearrange("p (a t) -> p a t", a=NCHUNK)
                    for a in range(NCHUNK):
                        nc.sync.dma_start(
                            tv[:, a, :], xt_r[:, a, w * W : (w + 1) * W]
                        )
                    xtw_tiles[w] = t
                return xtw_tiles[w]

            for a in range(NCHUNK):
                nc.sync.dma_start(
                    waqk_sb[a][:, 0:W], wqk[a * P : (a + 1) * P, 0:W]
                )
                if a == 0:
                    nc.sync.dma_start(bqk_t[:], bqk[:])
                    xtw_get(0)
            for a in range(NCHUNK):
                nc.sync.dma_start(
                    waqk_sb[a][:, W:C], wqk[a * P : (a + 1) * P, W:C]
                )
            for a in range(NCHUNK):
                nc.sync.dma_start(wav_sb[a][:], wv[a * P : (a + 1) * P, :])
            nc.sync.dma_start(bv_t[:], bv[:])
            nc.sync.dma_start(
                v_sb[:].rearrange("p (t c) -> p t c", c=65)[:, :, 64:65],
                onesd[:].rearrange("p (t c) -> p t c", c=1),
            )
            nc.sync.dma_start(ones_row[:], onesd[0:1, :])
            nc.sync.dma_start(tri_t[:], trimask[:])
            nc.sync.dma_start(tri2_t[:], trimask2[:])
            for ch in range(PAIRS):
                nc.sync.dma_start(wp_sb[ch][:], wp[ch * P : (ch + 1) * P, :])
            nc.sync.dma_start(bpr_t[:], bpr[:])

            qt_sb = [None] * PAIRS

            def emit_qk_copy(j, qk_ps, w):
                # move PSUM -> SBUF with the per-qk-column bias added
                if j < PAIRS:
                    qt_sb[j] = qt_pool.tile(
                        [P, W], F32R, tag=f"qt{j}", bufs=1, name=f"qt{j}_{w}"
                    )
                    dest = qt_sb[j][:]
                else:
                    dest = kt_sb[j - PAIRS][:, w * W : (w + 1) * W]
                nc.vector.tensor_scalar(
                    out=dest,
                    in0=qk_ps[:],
                    scalar1=bqk_t[:, j : j + 1],
                    scalar2=None,
                    op0=mybir.AluOpType.add,
                )

            def emit_v_add(i, v_ps, w):
                tb = 4 * w + i
                nc.vector.tensor_add(
                    v_view[:, :, tb, 0:D],
                    v_ps[:].rearrange("p (h d) -> p h d", h=HL),
                    bv_view[:, :, :],
                )

            def emit_qkv_window0():
                xtw = xtw_get(0)
                # chunk-major over 4-tile groups so PE can trail the DMA
                # stream chunk by chunk.
                for jg in range(2):  # Q pairs then K pairs
                    qk_ps = [
                        psum.tile([P, W], F32, tag="mm", name=f"qk0_{jg}{j}")
                        for j in range(4)
                    ]
                    for a in range(NCHUNK):
                        for j in range(4):
                            nc.tensor.matmul(
                                qk_ps[j][:],
                                waqk_sb[a][:, (4 * jg + j) * P : (4 * jg + j + 1) * P],
                                xtw[:, a * W : (a + 1) * W],
                                start=(a == 0),
                                stop=(a == NCHUNK - 1),
                            )
                    for j in range(4):
                        emit_qk_copy(4 * jg + j, qk_ps[j], 0)
                v_ps = [
                    psum.tile([P, W], F32, tag="mm", name=f"v0_{i}")
                    for i in range(4)
                ]
                for a in range(NCHUNK):
                    for i in range(4):
                        nc.tensor.matmul(
                            v_ps[i][:],
                            xtw[:, a * W + i * P : a * W + (i + 1) * P],
                            wav_sb[a][:],
                            start=(a == 0),
                            stop=(a == NCHUNK - 1),
                        )
                for i in range(4):
                    emit_v_add(i, v_ps[i], 0)

            def qkv_window_units(w):
                # windows >= 1: inputs already resident, j-major streaming.
                # Returns one closure per projection unit so the caller can
                # interleave them between attention heads as PE filler.
                xtw = xtw_get(w)

                def qk_unit(j):
                    def emit():
                        qk_ps = psum.tile([P, W], F32, tag="mm",
                                          name=f"qk{w}_{j}")
                        for a in range(NCHUNK):
                            nc.tensor.matmul(
                                qk_ps[:],
                                waqk_sb[a][:, j * P : (j + 1) * P],
                                xtw[:, a * W : (a + 1) * W],
                                start=(a == 0),
                                stop=(a == NCHUNK - 1),
                            )
                        emit_qk_copy(j, qk_ps, w)
                    return emit

                def v_unit(i):
                    def emit():
                        v_ps = psum.tile([P, W], F32, tag="mm",
                                         name=f"v{w}_{i}")
                        for a in range(NCHUNK):
                            nc.tensor.matmul(
                                v_ps[:],
                                xtw[:, a * W + i * P : a * W + (i + 1) * P],
                                wav_sb[a][:],
                                start=(a == 0),
                                stop=(a == NCHUNK - 1),
                            )
                        emit_v_add(i, v_ps, w)
                    return emit

                # per-head filler schedule: pair p's Q tile (bufs=1) is
                # only dead after head 2p+1 of the current window, so its
                # qk units may not be emitted earlier; V slots are disjoint.
                return {
                    0: [],
                    1: [qk_unit(0), qk_unit(4)],
                    2: [v_unit(0)],
                    3: [qk_unit(1), qk_unit(5)],
                    4: [v_unit(1)],
                    5: [qk_unit(2), qk_unit(6)],
                    6: [v_unit(2)],
                    7: [qk_unit(3), qk_unit(7), v_unit(3)],
                }

            emit_qkv_window0()

            def cproj_units(w, yt_tiles):
                # partial c_proj of a finished window's 4 token blocks, one
                # closure per (tb, ew) group so they can interleave as PE
                # filler inside the next window's attention.
                def unit(i, ew):
                    def emit():
                        tb = 4 * w + i
                        o_ps = psum.tile([P, W], F32, tag="mm",
                                         name=f"o{tb}_{ew}")
                        for ch in range(PAIRS):
                            nc.tensor.matmul(
                                o_ps[:],
                                yt_tiles[ch][:, i * P : (i + 1) * P],
                                wp_sb[ch][:, ew * W : (ew + 1) * W],
                                start=(ch == 0),
                                stop=(ch == PAIRS - 1),
                            )
                        o_sb = o_pool.tile([P, W], F32, tag="osb")
                        nc.vector.tensor_add(
                            o_sb[:], o_ps[:], bpr_t[:, ew * W : (ew + 1) * W]
                        )
                        nc.sync.dma_start(
                            out[tb * P : (tb + 1) * P, ew * W : (ew + 1) * W],
                            o_sb[:],
                        )
                    return emit
                return [unit(i, ew) for i in range(4) for ew in range(C // W)]

            yt_prev = None
            for w in range(NW):
                nkb = 4 * w + 4
                plan = {h: [] for h in range(HL)}
                if yt_prev is not None:
                    cps = cproj_units(w - 1, yt_prev)
                    for h, u in zip((0, 1, 2, 3, 4, 5, 6, 7), cps):
                        plan[h].append(u)
                if w + 1 < NW:
                    qkv = qkv_window_units(w + 1)
                    for h, us in qkv.items():
                        plan[h].extend(us)
                yt_w = [
                    yt_pool.tile([P, W], F32R, tag=f"yt{pr}", bufs=2,
                                 name=f"yt{pr}_{w}")
                    for pr in range(PAIRS)
                ]
                for h in range(HL):
                    pr, sub = h // 2, h % 2
                    QT = qt_sb[pr]
                    KT = kt_sb[pr]
                    y_ps = psum.tile([65, W], F32, tag="y", bufs=2,
                                     name=f"y{w}_{h}")
                    pending = []

                    def emit_scores(kb):
                        if kb < 4 * w:
                            cs, mk = 0, None
                        else:
                            i = kb - 4 * w
                            cs = (0, P, 2 * P, 2 * P)[i]
                            mk = i
                        s_ps = psum.tile([P, W], F32, tag="mm",
                                         name=f"s{w}_{h}_{kb}")
                        at = attn_pool.tile([P, W], F32R, tag="attn")
                        nc.tensor.matmul(
                            s_ps[:, cs:W],
                            KT[sub * D : (sub + 1) * D, kb * P : (kb + 1) * P],
                            QT[sub * D : (sub + 1) * D, cs:W],
                            start=True,
                            stop=True,
                        )
                        nc.scalar.activation(
                            at[:, cs:W], s_ps[:, cs:W], EXP,
                            scale=1.0 / np.sqrt(D),
                        )
                        if mk is not None:
                            if mk < 3:
                                nc.vector.tensor_mul(
                                    at[:, mk * P : (mk + 1) * P],
                                    at[:, mk * P : (mk + 1) * P],
                                    tri_t[:],
                                )
                            else:
                                nc.vector.tensor_mul(
                                    at[:, 2 * P : W],
                                    at[:, 2 * P : W],
                                    tri2_t[:],
                                )
                        return (kb, cs, at)

                    def emit_av(kb, cs, at):
                        nc.tensor.matmul(
                            y_ps[:, cs:W],
                            v_sb[:, (h * KB + kb) * 65 : (h * KB + kb + 1) * 65],
                            at[:, cs:W],
                            start=(kb == 0),
                            stop=(kb == nkb - 1),
                        )

                    for kb in range(nkb):
                        pending.append(emit_scores(kb))
                        if len(pending) > LAG:
                            emit_av(*pending.pop(0))
                    for item in pending:
                        emit_av(*item)

                    # softmax normalization: divide y rows by the denominator
                    # accumulated in PSUM row 64.
                    rc = norm_pool.tile([1, W], F32R, tag="recip",
                                        name=f"rc{w}_{h}")
                    nc.vector.reciprocal(rc[:], y_ps[64:65, :])
                    bc_ps = psum.tile([D, W], F32, tag="bc", bufs=2,
                                      name=f"bc{w}_{h}")
                    nc.tensor.matmul(
                        bc_ps[:], ones_row[0:1, 0:D], rc[:],
                        start=True, stop=True,
                    )
                    pbc = norm_pool.tile([D, W], F32, tag="pbc",
                                         name=f"pbc{w}_{h}")
                    nc.vector.tensor_copy(pbc[:], bc_ps[:])
                    nc.vector.tensor_mul(
                        yt_w[pr][sub * D : (sub + 1) * D, :],
                        y_ps[0:D, :],
                        pbc[:],
                    )

                    # interleaved PE filler: previous window's c_proj groups
                    # and next window's QKV units keep PE fed while Act
                    # works through this window's exp backlog.
                    for unit in plan[h]:
                        unit()

                yt_prev = yt_w

            for unit in cproj_units(NW - 1, yt_prev):
                unit()

    nc.compile()
    _CACHE["nc"] = nc
    return nc


def make_in_maps(x, w_attn, b_attn, w_proj, b_proj):
    """Host-side sharding: per-core input dict."""
    x = np.ascontiguousarray(np.asarray(x, dtype=np.float32))
    w_attn = np.asarray(w_attn, dtype=np.float32)
    b_attn = np.asarray(b_attn, dtype=np.float32)
    w_proj = np.asarray(w_proj, dtype=np.float32)
    b_proj = np.asarray(b_proj, dtype=np.float32)

    trimask = np.triu(np.ones((P, P), dtype=np.float32))  # [k, q]: 1 if q >= k
    trimask2 = np.concatenate(
        [np.zeros((P, P), dtype=np.float32), trimask], axis=1
    )
    in_maps = []
    for c in range(N_CORES):
        b = c // 2
        g = c % 2
        h0 = g * HL
        # Q/K columns arranged pair-wise: [q(h0) q(h0+1) | q(h0+2) ... | k(...)]
        qcols = np.arange(h0 * D, (h0 + HL) * D)
        kcols = C + qcols
        wqk = np.concatenate(
            [w_attn[:, qcols], w_attn[:, kcols]], axis=1
        )  # [C, 1024]
        bqk_flat = np.concatenate([b_attn[qcols], b_attn[kcols]])  # [1024]
        bqk = np.ascontiguousarray(bqk_flat.reshape(2 * PAIRS, P).T)  # [128, 8]
        vcols = 2 * C + np.arange(h0 * D, (h0 + HL) * D)
        wv = np.ascontiguousarray(w_attn[:, vcols])  # [C, 512]
        bv = np.broadcast_to(b_attn[vcols], (P, HL * D)).copy()
        wp = np.ascontiguousarray(w_proj[h0 * D : (h0 + HL) * D, :])  # [512, C]
        if g == 0:
            bpr = np.broadcast_to(b_proj, (P, C)).copy()
        else:
            bpr = np.zeros((P, C), dtype=np.float32)
        in_maps.append(
            {
                "xt": np.ascontiguousarray(x[b].T),  # [C, T]
                "wqk": wqk,
                "wv": wv,
                "bqk": bqk,
                "bv": bv,
                "wp": wp,
                "bpr": bpr,
                "trimask": trimask,
                "trimask2": trimask2,
                "onesd": np.ones((P, P), dtype=np.float32),
            }
        )
    return in_maps


def kernel(x, w_attn, b_attn, w_proj, b_proj, _trace=False):
    global LAST_RESULTS
    nc = build_nc()
    in_maps = make_in_maps(x, w_attn, b_attn, w_proj, b_proj)
    res = run_bass_kernel_spmd(
        nc, in_maps, list(range(N_CORES)), trace=_trace
    )
    LAST_RESULTS = res
    outs = [res.results[c]["out"] for c in range(N_CORES)]
    y = np.stack([outs[2 * b] + outs[2 * b + 1] for b in range(B)], axis=0)
    return y.astype(np.float32)


# revision 11
# speedup vs baseline: 1.1256x; 1.1256x over previous
"""Causal self-attention (B=4, T=2048, C=1024, H=16) on 8 TRN2 NeuronCores.

Sharding: hybrid batch x head tensor-parallel. Core c handles batch b = c//2
and heads [8*(c%2) : 8*(c%2)+8]. Each core computes QKV for its 8 heads over
its batch, full causal attention for those heads, and a *partial* c_proj
(contribution of its 8 heads to all 2048 tokens of its batch). The host
unshards by summing the two partial outputs of each batch pair; b_proj is
added on-device by the even core of each pair.

Single-pass pipelined structure (per core): one loop over the 4 q-windows of
512 tokens. Per window w: QK projections for that token window (Q kept only
for the window, K appended to a persistent K_T), V for the window's 4 token
blocks, then causal attention for all 8 heads over k-blocks 0..4w+3 (scores
matmul -> exp on Act -> triangular mask on DVE -> attn@V accumulate), per-head
softmax normalization (denominator rides in PSUM row 64 via a ones column in
the V tiles), then the window's partial c_proj with the bias folded in as a
rank-1 accumulate matmul and gpsimd moving PSUM->SBUF for the output DMA.
x is fed transposed (xt [C, T]) and loaded once per window; weights stay
resident in SBUF. All matmuls are float32r (full-rate fp32, moving dim kept
>= 256 everywhere: the 128-wide diagonal chunks are widened to 256 with a
zero-extended triangular mask).
"""

import numpy as np

import concourse.bass as bass
import concourse.mybir as mybir
import concourse.tile as tile
from concourse import bacc
from concourse.bass_utils import run_bass_kernel_spmd

B, T, C = 4, 2048, 1024
H = 16          # total heads
HL = 8          # heads per core
D = 64          # head dim
P = 128
W = 512         # q-window / matmul moving-dim size
NW = T // W     # 4 q windows
KB = T // P     # 16 k blocks
NCHUNK = C // P  # 8 contraction chunks over C
PAIRS = HL // 2  # 4 head-pairs (2 heads per 128-partition tile)
F32 = mybir.dt.float32
F32R = mybir.dt.float32r
EXP = mybir.ActivationFunctionType.Exp
N_CORES = 8
LAG = 2          # scores->attn@V software pipeline depth per head

_CACHE = {}
LAST_RESULTS = None


def build_nc():
    if "nc" in _CACHE:
        return _CACHE["nc"]
    nc = bacc.Bacc(
        "TRN2", target_bir_lowering=False, debug=False, num_devices=N_CORES
    )

    xt = nc.dram_tensor("xt", [C, T], F32R, kind="ExternalInput")
    wqk = nc.dram_tensor("wqk", [C, C], F32R, kind="ExternalInput")
    wv = nc.dram_tensor("wv", [C, HL * D], F32R, kind="ExternalInput")
    bqk = nc.dram_tensor("bqk", [P, 2 * PAIRS], F32, kind="ExternalInput")
    bv = nc.dram_tensor("bv", [P, HL * D], F32, kind="ExternalInput")
    wp = nc.dram_tensor("wp", [HL * D, C], F32R, kind="ExternalInput")
    bpr = nc.dram_tensor("bpr", [P, C], F32, kind="ExternalInput")
    trimask = nc.dram_tensor("trimask", [P, P], F32, kind="ExternalInput")
    trimask2 = nc.dram_tensor("trimask2", [P, 2 * P], F32, kind="ExternalInput")
    onesd = nc.dram_tensor("onesd", [P, P], F32R, kind="ExternalInput")
    out = nc.dram_tensor("out", [T, C], F32, kind="ExternalOutput")

    xt_r = xt[:].rearrange("(a p) t -> p a t", p=P)

    with tile.TileContext(nc) as tc, nc.allow_low_precision(
        reason="float32r tiles for full-rate fp32 PE matmuls"
    ):
        with (
            tc.tile_pool(name="consts", bufs=1) as consts,
            tc.tile_pool(name="waqk", bufs=NCHUNK) as waqk_pool,
            tc.tile_pool(name="wav", bufs=NCHUNK) as wav_pool,
            tc.tile_pool(name="xtw", bufs=1) as xtw_pool,
            tc.tile_pool(name="kt", bufs=1) as kt_pool,
            tc.tile_pool(name="qt", bufs=PAIRS) as qt_pool,
            tc.tile_pool(name="vsb", bufs=1) as v_pool,
            tc.tile_pool(name="attn", bufs=LAG + 1) as attn_pool,
            tc.tile_pool(name="yt", bufs=1) as yt_pool,
            tc.tile_pool(name="wp_sb", bufs=1) as wp_pool,
            tc.tile_pool(name="osb", bufs=2) as o_pool,
            tc.tile_pool(name="norm", bufs=1) as norm_pool,
            tc.tile_pool(name="psum", space="PSUM", bufs=3) as psum,
        ):
            # ---- const tiles
            bqk_t = consts.tile([P, 2 * PAIRS], F32)
            bv_t = consts.tile([P, HL * D], F32)
            tri_t = consts.tile([P, P], F32)
            tri2_t = consts.tile([P, 2 * P], F32)
            ones_row = consts.tile([1, P], F32R)
            bpr_t = consts.tile([P, C], F32)

            waqk_sb = [
                waqk_pool.tile([P, C], F32R, tag="waqk", name=f"waqk{a}")
                for a in range(NCHUNK)
            ]
            wav_sb = [
                wav_pool.tile([P, HL * D], F32R, tag="wav", name=f"wav{a}")
                for a in range(NCHUNK)
            ]
            kt_sb = [
                kt_pool.tile([P, T], F32R, tag=f"kt{pr}", name=f"kt{pr}")
                for pr in range(PAIRS)
            ]
            wp_sb = [
                wp_pool.tile([P, C], F32R, tag=f"wp{ch}", name=f"wp{ch}")
                for ch in range(PAIRS)
            ]
            # V laid out [tok, d] per (head, kblock) as [P, 65] slices
            # (col 64 stays 1.0 so attn@V accumulates softmax denominators).
            v_sb = v_pool.tile([P, HL * KB * 65], F32R)
            v_view = v_sb[:].rearrange("p (h k c) -> p h k c", h=HL, k=KB)
            bv_view = bv_t[:].rearrange("p (h d) -> p h d", h=HL)

            # ---- DMA kickoff, window-0 critical path first: Q-half weight
            # chunks interleaved with xt window-0 chunks, then K halves,
            # then wv; everything else after.
            xtw_tiles = {}

            def xtw_get(w):
                if w not in xtw_tiles:
                    t = xtw_pool.tile([P, NCHUNK * W], F32R, tag="xtw",
                                      name=f"xtw{w}")
                    tv = t[:].rearrange("p (a t) -> p a t", a=NCHUNK)
                    for a in range(NCHUNK):
                        nc.sync.dma_start(
                            tv[:, a, :], xt_r[:, a, w * W : (w + 1) * W]
                        )
                    xtw_tiles[w] = t
                return xtw_tiles[w]

            for a in range(NCHUNK):
                nc.sync.dma_start(
                    waqk_sb[a][:, 0:W], wqk[a * P : (a + 1) * P, 0:W]
                )
                if a == 0:
                    nc.sync.dma_start(bqk_t[:], bqk[:])
                    xtw_get(0)
            for a in range(NCHUNK):
                nc.sync.dma_start(
                    waqk_sb[a][:, W:C], wqk[a * P : (a + 1) * P, W:C]
                )
            for a in range(NCHUNK):
                nc.sync.dma_start(wav_sb[a][:], wv[a * P : (a + 1) * P, :])
            nc.sync.dma_start(bv_t[:], bv[:])
            nc.sync.dma_start(
                v_sb[:].rearrange("p (t c) -> p t c", c=65)[:, :, 64:65],
                onesd[:].rearrange("p (t c) -> p t c", c=1),
            )
            nc.sync.dma_start(ones_row[:], onesd[0:1, :])
            nc.sync.dma_start(tri_t[:], trimask[:])
            nc.sync.dma_start(tri2_t[:], trimask2[:])
            for ch in range(PAIRS):
                nc.sync.dma_start(wp_sb[ch][:], wp[ch * P : (ch + 1) * P, :])
            nc.sync.dma_start(bpr_t[:], bpr[:])

            qt_sb = [None] * PAIRS

            def emit_qk_copy(j, qk_ps, w):
                # move PSUM -> SBUF with the per-qk-column bias added
                if j < PAIRS:
                    qt_sb[j] = qt_pool.tile(
                        [P, W], F32R, tag=f"qt{j}", bufs=1, name=f"qt{j}_{w}"
                    )
                    dest = qt_sb[j][:]
                else:
                    dest = kt_sb[j - PAIRS][:, w * W : (w + 1) * W]
                nc.vector.tensor_scalar(
                    out=dest,
                    in0=qk_ps[:],
                    scalar1=bqk_t[:, j : j + 1],
                    scalar2=None,
                    op0=mybir.AluOpType.add,
                )

            def emit_v_add(i, v_ps, w):
                tb = 4 * w + i
                nc.vector.tensor_add(
                    v_view[:, :, tb, 0:D],
                    v_ps[:].rearrange("p (h d) -> p h d", h=HL),
                    bv_view[:, :, :],
                )

            def emit_qkv_window0():
                xtw = xtw_get(0)
                # chunk-major over 2-tile groups (fl banks) so PE can trail
                # the DMA stream chunk by chunk.
                for jg in range(4):
                    js = (jg, 4 + jg)
                    qk_ps = [
                        psum.tile([P, W], F32, tag="fl", bufs=2,
                                  name=f"qk0_{j}")
                        for j in js
                    ]
                    for a in range(NCHUNK):
                        for t, j in enumerate(js):
                            nc.tensor.matmul(
                                qk_ps[t][:],
                                waqk_sb[a][:, j * P : (j + 1) * P],
                                xtw[:, a * W : (a + 1) * W],
                                start=(a == 0),
                                stop=(a == NCHUNK - 1),
                            )
                    for t, j in enumerate(js):
                        emit_qk_copy(j, qk_ps[t], 0)
                for ig in range(2):
                    iis = (2 * ig, 2 * ig + 1)
                    v_ps = [
                        psum.tile([P, W], F32, tag="fl", bufs=2,
                                  name=f"v0_{i}")
                        for i in iis
                    ]
                    for a in range(NCHUNK):
                        for t, i in enumerate(iis):
                            nc.tensor.matmul(
                                v_ps[t][:],
                                xtw[:, a * W + i * P : a * W + (i + 1) * P],
                                wav_sb[a][:],
                                start=(a == 0),
                                stop=(a == NCHUNK - 1),
                            )
                    for t, i in enumerate(iis):
                        emit_v_add(i, v_ps[t], 0)

            def qkv_window_units(w):
                # windows >= 1: inputs already resident, j-major streaming.
                # Returns one closure per projection unit so the caller can
                # interleave them between attention heads as PE filler.
                xtw = xtw_get(w)

                def qk_unit(j):
                    def emit():
                        qk_ps = psum.tile([P, W], F32, tag="fl", bufs=2,
                                          name=f"qk{w}_{j}")
                        for a in range(NCHUNK):
                            nc.tensor.matmul(
                                qk_ps[:],
                                waqk_sb[a][:, j * P : (j + 1) * P],
                                xtw[:, a * W : (a + 1) * W],
                                start=(a == 0),
                                stop=(a == NCHUNK - 1),
                            )
                        emit_qk_copy(j, qk_ps, w)
                    return emit

                def v_unit(i):
                    def emit():
                        v_ps = psum.tile([P, W], F32, tag="fl", bufs=2,
                                         name=f"v{w}_{i}")
                        for a in range(NCHUNK):
                            nc.tensor.matmul(
                                v_ps[:],
                                xtw[:, a * W + i * P : a * W + (i + 1) * P],
                                wav_sb[a][:],
                                start=(a == 0),
                                stop=(a == NCHUNK - 1),
                            )
                        emit_v_add(i, v_ps, w)
                    return emit

                # per-head filler schedule: pair p's Q tile (bufs=1) is
                # only dead after head 2p+1 of the current window, so its
                # qk units may not be emitted earlier; V slots are disjoint.
                return {
                    0: [],
                    1: [qk_unit(0), qk_unit(4)],
                    2: [v_unit(0)],
                    3: [qk_unit(1), qk_unit(5)],
                    4: [v_unit(1)],
                    5: [qk_unit(2), qk_unit(6)],
                    6: [v_unit(2)],
                    7: [qk_unit(3), qk_unit(7), v_unit(3)],
                }

            emit_qkv_window0()

            def cproj_units(w, yt_tiles):
                # partial c_proj of a finished window's 4 token blocks, one
                # closure per (tb, ew) group so they can interleave as PE
                # filler inside the next window's attention.
                def unit(i, ew):
                    def emit():
                        tb = 4 * w + i
                        o_ps = psum.tile([P, W], F32, tag="fl", bufs=2,
                                         name=f"o{tb}_{ew}")
                        for ch in range(PAIRS):
                            nc.tensor.matmul(
                                o_ps[:],
                                yt_tiles[ch][:, i * P : (i + 1) * P],
                                wp_sb[ch][:, ew * W : (ew + 1) * W],
                                start=(ch == 0),
                                stop=(ch == PAIRS - 1),
                            )
                        o_sb = o_pool.tile([P, W], F32, tag="osb")
                        nc.vector.tensor_add(
                            o_sb[:], o_ps[:], bpr_t[:, ew * W : (ew + 1) * W]
                        )
                        nc.sync.dma_start(
                            out[tb * P : (tb + 1) * P, ew * W : (ew + 1) * W],
                            o_sb[:],
                        )
                    return emit
                return [unit(i, ew) for i in range(4) for ew in range(C // W)]

            yt_prev = None
            for w in range(NW):
                nkb = 4 * w + 4
                plan = {h: [] for h in range(HL)}
                if yt_prev is not None:
                    cps = cproj_units(w - 1, yt_prev)
                    for h, u in zip((0, 1, 2, 3, 4, 5, 6, 7), cps):
                        plan[h].append(u)
                if w + 1 < NW:
                    qkv = qkv_window_units(w + 1)
                    for h, us in qkv.items():
                        plan[h].extend(us)
                yt_w = [
                    yt_pool.tile([P, W], F32R, tag=f"yt{pr}", bufs=2,
                                 name=f"yt{pr}_{w}")
                    for pr in range(PAIRS)
                ]
                for h in range(HL):
                    pr, sub = h // 2, h % 2
                    QT = qt_sb[pr]
                    KT = kt_sb[pr]
                    y_ps = psum.tile([65, W], F32, tag="y", bufs=2,
                                     name=f"y{w}_{h}")
                    pending = []

                    def emit_scores(kb):
                        if kb < 4 * w:
                            cs, mk = 0, None
                        else:
                            i = kb - 4 * w
                            cs = (0, P, 2 * P, 2 * P)[i]
                            mk = i
                        s_ps = psum.tile([P, W], F32, tag="mm",
                                         name=f"s{w}_{h}_{kb}")
                        at = attn_pool.tile([P, W], F32R, tag="attn")
                        nc.tensor.matmul(
                            s_ps[:, cs:W],
                            KT[sub * D : (sub + 1) * D, kb * P : (kb + 1) * P],
                            QT[sub * D : (sub + 1) * D, cs:W],
                            start=True,
                            stop=True,
                        )
                        nc.scalar.activation(
                            at[:, cs:W], s_ps[:, cs:W], EXP,
                            scale=1.0 / np.sqrt(D),
                        )
                        if mk is not None:
                            if mk < 3:
                                nc.vector.tensor_mul(
                                    at[:, mk * P : (mk + 1) * P],
                                    at[:, mk * P : (mk + 1) * P],
                                    tri_t[:],
                                )
                            else:
                                nc.vector.tensor_mul(
                                    at[:, 2 * P : W],
                                    at[:, 2 * P : W],
                                    tri2_t[:],
                                )
                        return (kb, cs, at)

                    def emit_av(kb, cs, at):
                        nc.tensor.matmul(
                            y_ps[:, cs:W],
                            v_sb[:, (h * KB + kb) * 65 : (h * KB + kb + 1) * 65],
                            at[:, cs:W],
                            start=(kb == 0),
                            stop=(kb == nkb - 1),
                        )

                    for kb in range(nkb):
                        pending.append(emit_scores(kb))
                        if len(pending) > LAG:
                            emit_av(*pending.pop(0))
                    for item in pending:
                        emit_av(*item)

                    # softmax normalization: divide y rows by the denominator
                    # accumulated in PSUM row 64.
                    rc = norm_pool.tile([1, W], F32R, tag="recip",
                                        name=f"rc{w}_{h}")
                    nc.vector.reciprocal(rc[:], y_ps[64:65, :])
                    bc_ps = psum.tile([D, W], F32, tag="bc", bufs=1,
                                      name=f"bc{w}_{h}")
                    nc.tensor.matmul(
                        bc_ps[:], ones_row[0:1, 0:D], rc[:],
                        start=True, stop=True,
                    )
                    pbc = norm_pool.tile([D, W], F32, tag="pbc",
                                         name=f"pbc{w}_{h}")
                    nc.vector.tensor_copy(pbc[:], bc_ps[:])
                    nc.vector.tensor_mul(
                        yt_w[pr][sub * D : (sub + 1) * D, :],
                        y_ps[0:D, :],
                        pbc[:],
                    )

                    # interleaved PE filler: previous window's c_proj groups
                    # and next window's QKV units keep PE fed while Act
                    # works through this window's exp backlog.
                    for unit in plan[h]:
                        unit()

                yt_prev = yt_w

            for unit in cproj_units(NW - 1, yt_prev):
                unit()

    nc.compile()
    _CACHE["nc"] = nc
    return nc


def make_in_maps(x, w_attn, b_attn, w_proj, b_proj):
    """Host-side sharding: per-core input dict."""
    x = np.ascontiguousarray(np.asarray(x, dtype=np.float32))
    w_attn = np.asarray(w_attn, dtype=np.float32)
    b_attn = np.asarray(b_attn, dtype=np.float32)
    w_proj = np.asarray(w_proj, dtype=np.float32)
    b_proj = np.asarray(b_proj, dtype=np.float32)

    trimask = np.triu(np.ones((P, P), dtype=np.float32))  # [k, q]: 1 if q >= k
    trimask2 = np.concatenate(
        [np.zeros((P, P), dtype=np.float32), trimask], axis=1
    )
    in_maps = []
    for c in range(N_CORES):
        b = c // 2
        g = c % 2
        h0 = g * HL
        # Q/K columns arranged pair-wise: [q(h0) q(h0+1) | q(h0+2) ... | k(...)]
        qcols = np.arange(h0 * D, (h0 + HL) * D)
        kcols = C + qcols
        wqk = np.concatenate(
            [w_attn[:, qcols], w_attn[:, kcols]], axis=1
        )  # [C, 1024]
        bqk_flat = np.concatenate([b_attn[qcols], b_attn[kcols]])  # [1024]
        bqk = np.ascontiguousarray(bqk_flat.reshape(2 * PAIRS, P).T)  # [128, 8]
        vcols = 2 * C + np.arange(h0 * D, (h0 + HL) * D)
        wv = np.ascontiguousarray(w_attn[:, vcols])  # [C, 512]
        bv = np.broadcast_to(b_attn[vcols], (P, HL * D)).copy()
        wp = np.ascontiguousarray(w_proj[h0 * D : (h0 + HL) * D, :])  # [512, C]
        if g == 0:
            bpr = np.broadcast_to(b_proj, (P, C)).copy()
        else:
            bpr = np.zeros((P, C), dtype=np.float32)
        in_maps.append(
            {
                "xt": np.ascontiguousarray(x[b].T),  # [C, T]
                "wqk": wqk,
                "wv": wv,
                "bqk": bqk,
                "bv": bv,
                "wp": wp,
                "bpr": bpr,
                "trimask": trimask,
                "trimask2": trimask2,
                "onesd": np.ones((P, P), dtype=np.float32),
            }
        )
    return in_maps


def kernel(x, w_attn, b_attn, w_proj, b_proj, _trace=False):
    global LAST_RESULTS
    nc = build_nc()
    in_maps = make_in_maps(x, w_attn, b_attn, w_proj, b_proj)
    res = run_bass_kernel_spmd(
        nc, in_maps, list(range(N_CORES)), trace=_trace
    )
    LAST_RESULTS = res
    outs = [res.results[c]["out"] for c in range(N_CORES)]
    y = np.stack([outs[2 * b] + outs[2 * b + 1] for b in range(B)], axis=0)
    return y.astype(np.float32)


# revision 13
# speedup vs baseline: 1.1624x; 1.0327x over previous
"""Causal self-attention (B=4, T=2048, C=1024, H=16) on 8 TRN2 NeuronCores.

Sharding: hybrid batch x head tensor-parallel. Core c handles batch b = c//2
and heads [8*(c%2) : 8*(c%2)+8]. Each core computes QKV for its 8 heads over
its batch, full causal attention for those heads, and a *partial* c_proj
(contribution of its 8 heads to all 2048 tokens of its batch). The host
unshards by summing the two partial outputs of each batch pair; b_proj is
added on-device by the even core of each pair.

Single-pass pipelined structure (per core): one loop over the 4 q-windows of
512 tokens. Per window w: QK projections for that token window (Q kept only
for the window, K appended to a persistent K_T), V for the window's 4 token
blocks, then causal attention for all 8 heads over k-blocks 0..4w+3 (scores
matmul -> exp on Act -> triangular mask on DVE -> attn@V accumulate), per-head
softmax normalization (denominator rides in PSUM row 64 via a ones column in
the V tiles), then the window's partial c_proj with the bias folded in as a
rank-1 accumulate matmul and gpsimd moving PSUM->SBUF for the output DMA.
x is fed transposed (xt [C, T]) and loaded once per window; weights stay
resident in SBUF. All matmuls are float32r (full-rate fp32, moving dim kept
>= 256 everywhere: the 128-wide diagonal chunks are widened to 256 with a
zero-extended triangular mask).
"""

import numpy as np

import concourse.bass as bass
import concourse.mybir as mybir
import concourse.tile as tile
from concourse import bacc
from concourse.bass_utils import run_bass_kernel_spmd

B, T, C = 4, 2048, 1024
H = 16          # total heads
HL = 8          # heads per core
D = 64          # head dim
P = 128
W = 512         # q-window / matmul moving-dim size
NW = T // W     # 4 q windows
KB = T // P     # 16 k blocks
NCHUNK = C // P  # 8 contraction chunks over C
PAIRS = HL // 2  # 4 head-pairs (2 heads per 128-partition tile)
F32 = mybir.dt.float32
F32R = mybir.dt.float32r
EXP = mybir.ActivationFunctionType.Exp
N_CORES = 8
LAG = 2          # scores->attn@V software pipeline depth per head

_CACHE = {}
LAST_RESULTS = None


def build_nc():
    if "nc" in _CACHE:
        return _CACHE["nc"]
    nc = bacc.Bacc(
        "TRN2", target_bir_lowering=False, debug=False, num_devices=N_CORES
    )

    xt = nc.dram_tensor("xt", [C, T], F32R, kind="ExternalInput")
    wqk = nc.dram_tensor("wqk", [C, C], F32R, kind="ExternalInput")
    wv = nc.dram_tensor("wv", [C, HL * D], F32R, kind="ExternalInput")
    bqk = nc.dram_tensor("bqk", [P, 2 * PAIRS], F32, kind="ExternalInput")
    bv = nc.dram_tensor("bv", [P, HL * D], F32, kind="ExternalInput")
    wp = nc.dram_tensor("wp", [HL * D, C], F32R, kind="ExternalInput")
    bpr = nc.dram_tensor("bpr", [P, C], F32, kind="ExternalInput")
    trimask = nc.dram_tensor("trimask", [P, P], F32, kind="ExternalInput")
    trimask2 = nc.dram_tensor("trimask2", [P, 2 * P], F32, kind="ExternalInput")
    onesd = nc.dram_tensor("onesd", [P, P], F32R, kind="ExternalInput")
    out = nc.dram_tensor("out", [T, C], F32, kind="ExternalOutput")

    xt_r = xt[:].rearrange("(a p) t -> p a t", p=P)

    with tile.TileContext(nc) as tc, nc.allow_low_precision(
        reason="float32r tiles for full-rate fp32 PE matmuls"
    ):
        with (
            tc.tile_pool(name="consts", bufs=1) as consts,
            tc.tile_pool(name="waqk", bufs=NCHUNK) as waqk_pool,
            tc.tile_pool(name="wav", bufs=NCHUNK) as wav_pool,
            tc.tile_pool(name="xtw", bufs=1) as xtw_pool,
            tc.tile_pool(name="kt", bufs=1) as kt_pool,
            tc.tile_pool(name="qt", bufs=PAIRS) as qt_pool,
            tc.tile_pool(name="vsb", bufs=1) as v_pool,
            tc.tile_pool(name="attn", bufs=LAG + 1) as attn_pool,
            tc.tile_pool(name="yt", bufs=1) as yt_pool,
            tc.tile_pool(name="wp_sb", bufs=1) as wp_pool,
            tc.tile_pool(name="osb", bufs=2) as o_pool,
            tc.tile_pool(name="norm", bufs=1) as norm_pool,
            tc.tile_pool(name="psum", space="PSUM", bufs=3) as psum,
        ):
            # ---- const tiles
            bqk_t = consts.tile([P, 2 * PAIRS], F32)
            bv_t = consts.tile([P, HL * D], F32)
            tri_t = consts.tile([P, P], F32)
            tri2_t = consts.tile([P, 2 * P], F32)
            ones_row = consts.tile([1, P], F32R)
            bpr_t = consts.tile([P, C], F32)

            waqk_sb = [
                waqk_pool.tile([P, C], F32R, tag="waqk", name=f"waqk{a}")
                for a in range(NCHUNK)
            ]
            wav_sb = [
                wav_pool.tile([P, HL * D], F32R, tag="wav", name=f"wav{a}")
                for a in range(NCHUNK)
            ]
            kt_sb = [
                kt_pool.tile([P, T], F32R, tag=f"kt{pr}", name=f"kt{pr}")
                for pr in range(PAIRS)
            ]
            wp_sb = [
                wp_pool.tile([P, C], F32R, tag=f"wp{ch}", name=f"wp{ch}")
                for ch in range(PAIRS)
            ]
            # V laid out [tok, d] per (head, kblock) as [P, 65] slices
            # (col 64 stays 1.0 so attn@V accumulates softmax denominators).
            v_sb = v_pool.tile([P, HL * KB * 65], F32R)
            v_view = v_sb[:].rearrange("p (h k c) -> p h k c", h=HL, k=KB)
            bv_view = bv_t[:].rearrange("p (h d) -> p h d", h=HL)

            # ---- DMA kickoff, window-0 critical path first: Q-half weight
            # chunks interleaved with xt window-0 chunks, then K halves,
            # then wv; everything else after.
            xtw_tiles = {}

            def xtw_get(w):
                if w not in xtw_tiles:
                    t = xtw_pool.tile([P, NCHUNK * W], F32R, tag="xtw",
                                      name=f"xtw{w}")
                    tv = t[:].rearrange("p (a t) -> p a t", a=NCHUNK)
                    for a in range(NCHUNK):
                        nc.sync.dma_start(
                            tv[:, a, :], xt_r[:, a, w * W : (w + 1) * W]
                        )
                    xtw_tiles[w] = t
                return xtw_tiles[w]

            for a in range(NCHUNK):
                nc.sync.dma_start(
                    waqk_sb[a][:, 0:W], wqk[a * P : (a + 1) * P, 0:W]
                )
                if a == 0:
                    nc.sync.dma_start(bqk_t[:], bqk[:])
                    xtw_get(0)
            for a in range(NCHUNK):
                nc.sync.dma_start(
                    waqk_sb[a][:, W:C], wqk[a * P : (a + 1) * P, W:C]
                )
            for a in range(NCHUNK):
                nc.sync.dma_start(wav_sb[a][:], wv[a * P : (a + 1) * P, :])
            nc.sync.dma_start(bv_t[:], bv[:])
            nc.sync.dma_start(
                v_sb[:].rearrange("p (t c) -> p t c", c=65)[:, :, 64:65],
                onesd[:].rearrange("p (t c) -> p t c", c=1),
            )
            nc.sync.dma_start(ones_row[:], onesd[0:1, :])
            nc.sync.dma_start(tri_t[:], trimask[:])
            nc.sync.dma_start(tri2_t[:], trimask2[:])
            for ch in range(PAIRS):
                nc.sync.dma_start(wp_sb[ch][:], wp[ch * P : (ch + 1) * P, :])
            nc.sync.dma_start(bpr_t[:], bpr[:])

            qt_sb = [None] * PAIRS

            def emit_qk_copy(j, qk_ps, w):
                # move PSUM -> SBUF with the per-qk-column bias added
                if j < PAIRS:
                    qt_sb[j] = qt_pool.tile(
                        [P, W], F32R, tag=f"qt{j}", bufs=1, name=f"qt{j}_{w}"
                    )
                    dest = qt_sb[j][:]
                else:
                    dest = kt_sb[j - PAIRS][:, w * W : (w + 1) * W]
                nc.vector.tensor_scalar(
                    out=dest,
                    in0=qk_ps[:],
                    scalar1=bqk_t[:, j : j + 1],
                    scalar2=None,
                    op0=mybir.AluOpType.add,
                )

            def emit_v_add(i, v_ps, w):
                tb = 4 * w + i
                nc.vector.tensor_add(
                    v_view[:, :, tb, 0:D],
                    v_ps[:].rearrange("p (h d) -> p h d", h=HL),
                    bv_view[:, :, :],
                )

            def emit_qkv_window0():
                xtw = xtw_get(0)
                # chunk-major over 2-tile groups (fl banks) so PE can trail
                # the DMA stream chunk by chunk.
                for jg in range(4):
                    js = (jg, 4 + jg)
                    qk_ps = [
                        psum.tile([P, W], F32, tag="fl", bufs=2,
                                  name=f"qk0_{j}")
                        for j in js
                    ]
                    for a in range(NCHUNK):
                        for t, j in enumerate(js):
                            nc.tensor.matmul(
                                qk_ps[t][:],
                                waqk_sb[a][:, j * P : (j + 1) * P],
                                xtw[:, a * W : (a + 1) * W],
                                start=(a == 0),
                                stop=(a == NCHUNK - 1),
                            )
                    for t, j in enumerate(js):
                        emit_qk_copy(j, qk_ps[t], 0)
                for ig in range(2):
                    iis = (2 * ig, 2 * ig + 1)
                    v_ps = [
                        psum.tile([P, W], F32, tag="fl", bufs=2,
                                  name=f"v0_{i}")
                        for i in iis
                    ]
                    for a in range(NCHUNK):
                        for t, i in enumerate(iis):
                            nc.tensor.matmul(
                                v_ps[t][:],
                                xtw[:, a * W + i * P : a * W + (i + 1) * P],
                                wav_sb[a][:],
                                start=(a == 0),
                                stop=(a == NCHUNK - 1),
                            )
                    for t, i in enumerate(iis):
                        emit_v_add(i, v_ps[t], 0)

            def qkv_window_units(w):
                # windows >= 1: inputs already resident, j-major streaming.
                # Returns one closure per projection unit so the caller can
                # interleave them between attention heads as PE filler.
                xtw = xtw_get(w)

                def qk_unit(j):
                    def emit():
                        qk_ps = psum.tile([P, W], F32, tag="fl", bufs=2,
                                          name=f"qk{w}_{j}")
                        for a in range(NCHUNK):
                            nc.tensor.matmul(
                                qk_ps[:],
                                waqk_sb[a][:, j * P : (j + 1) * P],
                                xtw[:, a * W : (a + 1) * W],
                                start=(a == 0),
                                stop=(a == NCHUNK - 1),
                            )
                        emit_qk_copy(j, qk_ps, w)
                    return emit

                def v_unit(i):
                    def emit():
                        v_ps = psum.tile([P, W], F32, tag="fl", bufs=2,
                                         name=f"v{w}_{i}")
                        for a in range(NCHUNK):
                            nc.tensor.matmul(
                                v_ps[:],
                                xtw[:, a * W + i * P : a * W + (i + 1) * P],
                                wav_sb[a][:],
                                start=(a == 0),
                                stop=(a == NCHUNK - 1),
                            )
                        emit_v_add(i, v_ps, w)
                    return emit

                # per-head filler schedule: pair p's Q tile (bufs=1) is
                # only dead after head 2p+1 of the current window, so its
                # qk units may not be emitted earlier; V slots are disjoint.
                return {
                    0: [],
                    1: [qk_unit(0), qk_unit(4)],
                    2: [v_unit(0)],
                    3: [qk_unit(1), qk_unit(5)],
                    4: [v_unit(1)],
                    5: [qk_unit(2), qk_unit(6)],
                    6: [v_unit(2)],
                    7: [qk_unit(3), qk_unit(7), v_unit(3)],
                    "flat": [qk_unit(j) for j in range(2 * PAIRS)]
                    + [v_unit(i) for i in range(4)],
                }

            emit_qkv_window0()

            def cproj_units(w, yt_tiles):
                # partial c_proj of a finished window's 4 token blocks, one
                # closure per (tb, ew) group so they can interleave as PE
                # filler inside the next window's attention.
                def unit(i, ew):
                    def emit():
                        tb = 4 * w + i
                        o_ps = psum.tile([P, W], F32, tag="fl", bufs=2,
                                         name=f"o{tb}_{ew}")
                        for ch in range(PAIRS):
                            nc.tensor.matmul(
                                o_ps[:],
                                yt_tiles[ch][:, i * P : (i + 1) * P],
                                wp_sb[ch][:, ew * W : (ew + 1) * W],
                                start=(ch == 0),
                                stop=(ch == PAIRS - 1),
                            )
                        o_sb = o_pool.tile([P, W], F32, tag="osb")
                        nc.vector.tensor_add(
                            o_sb[:], o_ps[:], bpr_t[:, ew * W : (ew + 1) * W]
                        )
                        nc.sync.dma_start(
                            out[tb * P : (tb + 1) * P, ew * W : (ew + 1) * W],
                            o_sb[:],
                        )
                    return emit
                return [unit(i, ew) for i in range(4) for ew in range(C // W)]

            yt_prev = None
            carry = {}  # units deferred into the NEXT window's plan
            for w in range(NW):
                nkb = 4 * w + 4
                plan = {h: [] for h in range(HL)}
                for h, us in carry.items():
                    plan[h].extend(us)
                carry = {}
                if yt_prev is not None:
                    cps = cproj_units(w - 1, yt_prev)
                    for h, u in zip((0, 1, 2, 3, 4, 5, 6, 7), cps):
                        plan[h].append(u)
                if w + 1 < NW:
                    qkv = qkv_window_units(w + 1)
                    if w + 1 == NW - 1:
                        # Rebalance for the Act-bound last window: its qk
                        # units run just before the pair that needs them,
                        # giving PE filler where Act is the bottleneck. V
                        # units stay in this window (all V blocks are
                        # needed by every last-window head).
                        qf = {j: u for j, u in enumerate(qkv.pop("flat"))}
                        plan[5].extend([qf[8], qf[9]])     # v0, v1
                        plan[6].extend([qf[10], qf[11]])   # v2, v3
                        plan[7].extend([qf[0], qf[4]])     # qk pair 0
                        carry = {
                            1: [qf[1], qf[5]],
                            3: [qf[2], qf[6]],
                            5: [qf[3], qf[7]],
                        }
                    else:
                        for h, us in qkv.items():
                            if h != "flat":
                                plan[h].extend(us)
                yt_w = [
                    yt_pool.tile([P, W], F32R, tag=f"yt{pr}", bufs=2,
                                 name=f"yt{pr}_{w}")
                    for pr in range(PAIRS)
                ]
                for h in range(HL):
                    pr, sub = h // 2, h % 2
                    QT = qt_sb[pr]
                    KT = kt_sb[pr]
                    y_ps = psum.tile([65, W], F32, tag="y", bufs=2,
                                     name=f"y{w}_{h}")
                    pending = []

                    def emit_scores(kb):
                        if kb < 4 * w:
                            cs, mk = 0, None
                        else:
                            i = kb - 4 * w
                            cs = (0, P, 2 * P, 2 * P)[i]
                            mk = i
                        s_ps = psum.tile([P, W], F32, tag="mm",
                                         name=f"s{w}_{h}_{kb}")
                        at = attn_pool.tile([P, W], F32R, tag="attn")
                        nc.tensor.matmul(
                            s_ps[:, cs:W],
                            KT[sub * D : (sub + 1) * D, kb * P : (kb + 1) * P],
                            QT[sub * D : (sub + 1) * D, cs:W],
                            start=True,
                            stop=True,
                        )
                        nc.scalar.activation(
                            at[:, cs:W], s_ps[:, cs:W], EXP,
                            scale=1.0 / np.sqrt(D),
                        )
                        if mk is not None:
                            if mk < 3:
                                nc.vector.tensor_mul(
                                    at[:, mk * P : (mk + 1) * P],
                                    at[:, mk * P : (mk + 1) * P],
                                    tri_t[:],
                                )
                            else:
                                nc.vector.tensor_mul(
                                    at[:, 2 * P : W],
                                    at[:, 2 * P : W],
                                    tri2_t[:],
                                )
                        return (kb, cs, at)

                    def emit_av(kb, cs, at):
                        nc.tensor.matmul(
                            y_ps[:, cs:W],
                            v_sb[:, (h * KB + kb) * 65 : (h * KB + kb + 1) * 65],
                            at[:, cs:W],
                            start=(kb == 0),
                            stop=(kb == nkb - 1),
                        )

                    for kb in range(nkb):
                        pending.append(emit_scores(kb))
                        if len(pending) > LAG:
                            emit_av(*pending.pop(0))
                    for item in pending:
                        emit_av(*item)

                    # softmax normalization: divide y rows by the denominator
                    # accumulated in PSUM row 64.
                    rc = norm_pool.tile([1, W], F32R, tag="recip",
                                        name=f"rc{w}_{h}")
                    nc.vector.reciprocal(rc[:], y_ps[64:65, :])
                    bc_ps = psum.tile([D, W], F32, tag="bc", bufs=1,
                                      name=f"bc{w}_{h}")
                    nc.tensor.matmul(
                        bc_ps[:], ones_row[0:1, 0:D], rc[:],
                        start=True, stop=True,
                    )
                    pbc = norm_pool.tile([D, W], F32, tag="pbc",
                                         name=f"pbc{w}_{h}")
                    nc.vector.tensor_copy(pbc[:], bc_ps[:])
                    nc.vector.tensor_mul(
                        yt_w[pr][sub * D : (sub + 1) * D, :],
                        y_ps[0:D, :],
                        pbc[:],
                    )

                    # interleaved PE filler: previous window's c_proj groups
                    # and next window's QKV units keep PE fed while Act
                    # works through this window's exp backlog.
                    for unit in plan[h]:
                        unit()

                yt_prev = yt_w

            for unit in cproj_units(NW - 1, yt_prev):
                unit()

    nc.compile()
    _CACHE["nc"] = nc
    return nc


def make_in_maps(x, w_attn, b_attn, w_proj, b_proj):
    """Host-side sharding: per-core input dict."""
    x = np.ascontiguousarray(np.asarray(x, dtype=np.float32))
    w_attn = np.asarray(w_attn, dtype=np.float32)
    b_attn = np.asarray(b_attn, dtype=np.float32)
    w_proj = np.asarray(w_proj, dtype=np.float32)
    b_proj = np.asarray(b_proj, dtype=np.float32)

    trimask = np.triu(np.ones((P, P), dtype=np.float32))  # [k, q]: 1 if q >= k
    trimask2 = np.concatenate(
        [np.zeros((P, P), dtype=np.float32), trimask], axis=1
    )
    in_maps = []
    for c in range(N_CORES):
        b = c // 2
        g = c % 2
        h0 = g * HL
        # Q/K columns arranged pair-wise: [q(h0) q(h0+1) | q(h0+2) ... | k(...)]
        qcols = np.arange(h0 * D, (h0 + HL) * D)
        kcols = C + qcols
        wqk = np.concatenate(
            [w_attn[:, qcols], w_attn[:, kcols]], axis=1
        )  # [C, 1024]
        bqk_flat = np.concatenate([b_attn[qcols], b_attn[kcols]])  # [1024]
        bqk = np.ascontiguousarray(bqk_flat.reshape(2 * PAIRS, P).T)  # [128, 8]
        vcols = 2 * C + np.arange(h0 * D, (h0 + HL) * D)
        wv = np.ascontiguousarray(w_attn[:, vcols])  # [C, 512]
        bv = np.broadcast_to(b_attn[vcols], (P, HL * D)).copy()
        wp = np.ascontiguousarray(w_proj[h0 * D : (h0 + HL) * D, :])  # [512, C]
        if g == 0:
            bpr = np.broadcast_to(b_proj, (P, C)).copy()
        else:
            bpr = np.zeros((P, C), dtype=np.float32)
        in_maps.append(
            {
                "xt": np.ascontiguousarray(x[b].T),  # [C, T]
                "wqk": wqk,
                "wv": wv,
                "bqk": bqk,
                "bv": bv,
                "wp": wp,
                "bpr": bpr,
                "trimask": trimask,
                "trimask2": trimask2,
                "onesd": np.ones((P, P), dtype=np.float32),
            }
        )
    return in_maps


def kernel(x, w_attn, b_attn, w_proj, b_proj, _trace=False):
    global LAST_RESULTS
    nc = build_nc()
    in_maps = make_in_maps(x, w_attn, b_attn, w_proj, b_proj)
    res = run_bass_kernel_spmd(
        nc, in_maps, list(range(N_CORES)), trace=_trace
    )
    LAST_RESULTS = res
    outs = [res.results[c]["out"] for c in range(N_CORES)]
    y = np.stack([outs[2 * b] + outs[2 * b + 1] for b in range(B)], axis=0)
    return y.astype(np.float32)
